# revision 68
# baseline (speedup 1.0000x reference)
"""Trainium2 Bass kernel for nn_CAM: channel attention (CAM) block.

y = gamma * gelu(conv3x3(attn(x))) + x   with
  q/k/v = 1x1 conv projections (d = C/8 = 32),
  energy[d,e] = sum_n q[d,n] k[e,n]  (n over all H*W positions),
  attn = softmax(max_e(energy) - energy, axis=e)  (== softmax(-energy)),
  out  = attn @ v.

Sharding: 8 cores, 2 per sample (B=4). Each core handles 64 rows of H plus
one halo row. Bottom-half cores receive a vertically flipped tile (and a
dy-flipped conv weight) so the SPMD program is identical on all cores; the
energy partial sums are combined with a pairwise AllReduce (4 KB).

Kernel pipeline (per core), all bf16 on-chip except f32 PSUM/softmax:
  x bf16 (host pre-cast) -> SBUF                       [4.26 MB HBM load]
  qk^T chunks = x_chunk(stationary).T @ [wq|wk]        [Q^T/K^T direct --
    no DMA transpose; biases added in the PSUM evac via a host-broadcast
    constant, so the energy is exact with no correction terms]
  E[32,32] = sum_chunks Q'^T K'  (exact biased energy, f32 PSUM accum)
  E -> pairwise AllReduce;  V = wv.T @ x + bv -> pa3 center block plus
    two flat +-1-shifted replica copies (SBUF DMAs) DURING the CC window
  softmax(-E) -> attn;  wp' = attn.T @ wp  (fold attention into the conv
    weights; conv then reads the V replicas directly -- no attn@V pass)
  conv3x3 = 3 accumulating K=96 bf16 matmuls per [128,1024] tile,
    dy-major so each weight stationary serves 2 consecutive matmuls
  y = gamma * gelu(conv) + x  (ACT gelu + DVE STT), bf16 store
  (host assembles and upcasts to f32)
"""
import sys

sys.path.insert(0, "/opt/trn_rl_repo")

from contextlib import ExitStack

import numpy as np
import ml_dtypes

import jax
from jax.sharding import Mesh, PartitionSpec, NamedSharding
from jax.experimental.shard_map import shard_map

import concourse.bacc as bacc
import concourse.tile as tile
from concourse import mybir
import concourse.bass as bass
from concourse.bass2jax import (
    _bass_exec_p,
    install_neuronx_cc_hook,
    partition_id_tensor,
)

F32 = mybir.dt.float32
BF16 = mybir.dt.bfloat16
OP = mybir.AluOpType
AF = mybir.ActivationFunctionType

C = 256
D = 32
H = 128
W = 128
HE = 65          # rows per core incl. 1 halo row
NE = HE * W      # 8320
NOWN = 64 * W    # 8192 (rows owned by this core)
NCH = 65         # 128-col chunks incl. halo row chunk
N_CORES = 8
REPLICA_GROUPS = [[0, 1], [2, 3], [4, 5], [6, 7]]


def make_pools(tc, _ctx, big_bufs=1):
    return dict(
        consts=_ctx.enter_context(tc.tile_pool(name="consts", bufs=1)),
        big=_ctx.enter_context(tc.tile_pool(name="big", bufs=big_bufs)),
        work=_ctx.enter_context(tc.tile_pool(name="work", bufs=4)),
        small=_ctx.enter_context(tc.tile_pool(name="small", bufs=2)),
        ps_qk=_ctx.enter_context(tc.tile_pool(name="ps_qk", bufs=3, space="PSUM")),
        ps_e=_ctx.enter_context(tc.tile_pool(name="ps_e", bufs=1, space="PSUM")),
        ps_c=_ctx.enter_context(tc.tile_pool(name="ps_c", bufs=2, space="PSUM")),
        dram=_ctx.enter_context(tc.tile_pool(name="dram", bufs=1, space="DRAM")),
    )


def build_consts(tc, aps, pools):
    """Load all loop-invariant constants once, outside the For_i body."""
    nc = tc.nc
    wqkT, wvT, bqk8, bvv, wpf, gam = (
        aps["wqkT"], aps["wvT"], aps["bqk8"], aps["bv"], aps["wpf"],
        aps["gamma"],
    )
    consts = pools["consts"]
    wqk_sb = consts.tile([128, 2, 64], BF16, tag="wqk")
    wv_sb = consts.tile([128, 2, 32], BF16, tag="wv")
    for c in range(2):
        nc.gpsimd.dma_start(out=wqk_sb[:, c, :], in_=wqkT[c])
        nc.gpsimd.dma_start(out=wv_sb[:, c, :], in_=wvT[c])
    bqk8_sb = consts.tile([128, 8, 64], BF16, tag="bqk8")
    nc.gpsimd.dma_start(out=bqk8_sb[:].rearrange("p a b -> p (a b)"), in_=bqk8)
    bv_sb = consts.tile([32, 1], F32)
    nc.gpsimd.dma_start(
        out=bv_sb[:],
        in_=bass.AP(tensor=bvv.tensor, offset=bvv.offset, ap=[[1, 32], [1, 1]]))
    wpf_sb = consts.tile([32, 3, 768], BF16, tag="wpf")
    for dx in range(3):
        nc.gpsimd.dma_start(out=wpf_sb[:, dx, :], in_=wpf[dx])
    gam_sb = consts.tile([128, 1], BF16)
    nc.gpsimd.dma_start(
        out=gam_sb[:],
        in_=bass.AP(tensor=gam.tensor, offset=gam.offset, ap=[[0, 128], [1, 1]]))
    return dict(wqk_sb=wqk_sb, wv_sb=wv_sb, bqk8_sb=bqk8_sb, bv_sb=bv_sb,
                wpf_sb=wpf_sb, gam_sb=gam_sb)


def build_body(tc, aps, pools, cst, use_cc=True):
    nc = tc.nc
    xe, y = aps["xe"], aps["y"]
    y_f = y.rearrange("c h w -> c (h w)")            # [256, 8192] bf16

    big, work, small = pools["big"], pools["work"], pools["small"]
    ps_qk, ps_e, ps_c, dram = (pools["ps_qk"], pools["ps_e"], pools["ps_c"],
                               pools["dram"])
    wqk_sb, wv_sb, bqk8_sb, bv_sb, wpf_sb, gam_sb = (
        cst["wqk_sb"], cst["wv_sb"], cst["bqk8_sb"], cst["bv_sb"],
        cst["wpf_sb"], cst["gam_sb"])

    # ---- big SBUF tensors ----
    x_sb = big.tile([128, 2, NE], BF16)              # both channel halves
    qk_sb = big.tile([128, 64, 64], BF16)            # [n128, chunk, q|k]
    pa3 = big.tile([96, 66, 130], BF16)              # V replicated w-shifted x3

    # ---- x load (bf16, sync + scalar queues), 8 groups of 1024 cols ----
    for g in range(8):
        s = g * 1024
        nc.sync.dma_start(out=x_sb[:, 0, s:s + 1024], in_=xe[0, :, s:s + 1024])
        nc.scalar.dma_start(out=x_sb[:, 1, s:s + 1024], in_=xe[1, :, s:s + 1024])
    nc.sync.dma_start(out=x_sb[:, 0, 8192:NE], in_=xe[0, :, 8192:NE])
    nc.scalar.dma_start(out=x_sb[:, 1, 8192:NE], in_=xe[1, :, 8192:NE])

    # pa3 zero padding: top row (h=-1) + center-block pad columns
    nc.vector.memset(pa3[:, 0, :], 0.0)
    nc.vector.memset(pa3[32:64, :, 0], 0.0)
    nc.vector.memset(pa3[32:64, :, 129], 0.0)

    # ---- qk^T chunks + energy accumulation ----
    # qk^T[n, ch] for chunk k: x_chunk [128c, 128n] stationary, wqk streamed;
    # biases folded in during the PSUM evac via a host-broadcast constant.
    e_ps = ps_e.tile([32, 32], F32, tag="eps")
    for g in range(8):
        qp = ps_qk.tile([128, 512], F32, tag="qk")
        for k in range(8):
            ch = 8 * g + k
            sl = slice(ch * 128, ch * 128 + 128)
            nc.tensor.matmul(qp[:, 64 * k:64 * k + 64], x_sb[:, 0, sl],
                             wqk_sb[:, 0, :], start=True, stop=False)
            nc.tensor.matmul(qp[:, 64 * k:64 * k + 64], x_sb[:, 1, sl],
                             wqk_sb[:, 1, :], start=False, stop=True)
        nc.vector.tensor_tensor(
            out=qk_sb[:, 8 * g:8 * g + 8, :],
            in0=qp[:].rearrange("p (k c) -> p k c", c=64),
            in1=bqk8_sb[:], op=OP.add)
        for k in range(8):
            ch = 8 * g + k
            nc.tensor.matmul(e_ps[:], qk_sb[:, ch, 0:32],
                             qk_sb[:, ch, 32:64],
                             start=(ch == 0), stop=(ch == 63))

    e_sb = small.tile([32, 32], F32, tag="esb")
    nc.vector.tensor_copy(out=e_sb[:], in_=e_ps[:])

    # ---- V projection -> pa3 center block (fills the CC window) ----
    nv = (NE + 511) // 512  # 17
    for i in range(nv):
        s = i * 512
        w = min(512, NE - s)
        nh = w // 128
        r0 = s // 128
        vp = ps_qk.tile([32, 512], F32, tag="qk")
        nc.tensor.matmul(vp[:, :w], wv_sb[:, 0, :], x_sb[:, 0, s:s + w],
                         start=True, stop=False)
        nc.tensor.matmul(vp[:, :w], wv_sb[:, 1, :], x_sb[:, 1, s:s + w],
                         start=False, stop=True)
        dst = pa3[32:64, 1 + r0:1 + r0 + nh, 1:129]
        src = vp[:, :w].rearrange("p (h w) -> p h w", w=128)
        if i % 2 == 0:
            nc.vector.tensor_scalar(out=dst, in0=src, scalar1=bv_sb[:],
                                    scalar2=None, op0=OP.add)
        else:
            nc.scalar.activation(out=dst, in_=src, func=AF.Identity,
                                 bias=bv_sb[:], scale=1.0)

    # ---- AllReduce energy across the sample pair ----
    E_sb = small.tile([32, 32], F32, tag="Esb")
    if use_cc:
        ein = dram.tile([32, 32], F32)
        eout = dram.tile([32, 32], F32)
        nc.gpsimd.dma_start(out=ein[:], in_=e_sb[:])
        nc.gpsimd.collective_compute(
            "AllReduce", OP.add, replica_groups=REPLICA_GROUPS,
            ins=[ein.opt()], outs=[eout.opt()])
        nc.sync.dma_start(out=E_sb[:], in_=eout[:])
    else:
        nc.gpsimd.tensor_copy(out=E_sb[:], in_=e_sb[:])

    # replicate the two w-shifted blocks as flat +-1-element shifted copies
    # (the zero pad columns make the row-boundary bleed exactly correct);
    # quartered so each segment launches as soon as its V rows land
    ctr = pa3[32:64, :, :].rearrange("p a b -> p (a b)")
    bl0 = pa3[0:32, :, :].rearrange("p a b -> p (a b)")
    bl2 = pa3[64:96, :, :].rearrange("p a b -> p (a b)")
    for r0, r1 in ((0, 17), (17, 34), (34, 50), (50, 66)):
        a, b = r0 * 130, r1 * 130
        if r1 == 66:
            b = 8579
        nc.sync.dma_start(out=bl0[:, a + 1:b + 1], in_=ctr[:, a:b])
        nc.scalar.dma_start(out=bl2[:, a:b], in_=ctr[:, a + 1:b + 1])

    # ---- softmax over e of -E, stable via min ----
    rmin = small.tile([32, 1], F32, tag="rmin")
    nc.vector.tensor_reduce(out=rmin[:], in_=E_sb[:], axis=mybir.AxisListType.X,
                            op=OP.min)
    p_sb = small.tile([32, 32], F32, tag="psb")
    nc.scalar.activation(out=p_sb[:], in_=E_sb[:], func=AF.Exp, scale=-1.0,
                         bias=rmin[:])
    ssum = small.tile([32, 1], F32, tag="ssum")
    nc.vector.reduce_sum(out=ssum[:], in_=p_sb[:], axis=mybir.AxisListType.X)
    rs = small.tile([32, 1], F32, tag="rs")
    nc.vector.reciprocal(out=rs[:], in_=ssum[:])
    attn_sb = small.tile([32, 32], BF16, tag="attn")
    nc.vector.tensor_scalar(out=attn_sb[:], in0=p_sb[:], scalar1=rs[:],
                            scalar2=None, op0=OP.mult)

    # ---- fold attention into conv weights: wp'[(dx,e),(dy,c)] ----
    wpp_sb = small.tile([96, 3, 256], BF16, tag="wpp")
    for dx in range(3):
        fp = ps_c.tile([32, 768], F32, tag="mm")
        nc.tensor.matmul(fp[:, 0:512], attn_sb[:], wpf_sb[:, dx, 0:512],
                         start=True, stop=True)
        nc.tensor.matmul(fp[:, 512:768], attn_sb[:], wpf_sb[:, dx, 512:768],
                         start=True, stop=True)
        dst = wpp_sb[32 * dx:32 * dx + 32, :, :].rearrange("p a b -> p (a b)")
        if dx == 1:
            nc.scalar.activation(out=dst, in_=fp[:], func=AF.Copy)
        else:
            nc.vector.tensor_copy(out=dst, in_=fp[:])

    # ---- conv 3x3 (bf16) + exact gelu + gamma*out + x, then store ----
    # dy-major matmul order within each [128,1024] tile: each conv weight
    # stationary is reused for 2 consecutive 512-col matmuls.
    for half in range(2):
        for tg in range(8):
            cp = ps_c.tile([128, 1024], F32, tag="mm")
            for dy in range(3):
                for tq in range(2):
                    t = 2 * tg + tq
                    nc.tensor.matmul(
                        cp[:, 512 * tq:512 * (tq + 1)],
                        wpp_sb[:, dy, half * 128:(half + 1) * 128],
                        pa3[:, 4 * t + dy:4 * t + dy + 4, 1:129],
                        start=(dy == 0), stop=(dy == 2))
            yt = work.tile([128, 1024], BF16, tag="yt")
            nc.scalar.activation(out=yt[:], in_=cp[:], func=AF.Gelu)
            t0 = 2 * tg * 512
            yo = work.tile([128, 1024], BF16, tag="yo")
            nc.vector.scalar_tensor_tensor(
                out=yo[:], in0=yt[:], scalar=gam_sb[:],
                in1=x_sb[:, half, t0:t0 + 1024],
                op0=OP.mult, op1=OP.add)
            st_eng = nc.sync if tg % 2 == 0 else nc.gpsimd
            st_eng.dma_start(
                out=y_f[half * 128:(half + 1) * 128, t0:t0 + 1024], in_=yo[:])


def build_nc(loop_k=None, use_cc=True, trace_sim=False, static_k=1):
    nc = bacc.Bacc("TRN2", target_bir_lowering=False, debug=False,
                   num_devices=N_CORES)
    aps = {
        "xe": nc.dram_tensor("xe", [2, 128, NE], BF16, kind="ExternalInput").ap(),
        "wqkT": nc.dram_tensor("wqkT", [2, 128, 64], BF16, kind="ExternalInput").ap(),
        "wvT": nc.dram_tensor("wvT", [2, 128, 32], BF16, kind="ExternalInput").ap(),
        "bqk8": nc.dram_tensor("bqk8", [128, 512], BF16, kind="ExternalInput").ap(),
        "bv": nc.dram_tensor("bv", [D], F32, kind="ExternalInput").ap(),
        "wpf": nc.dram_tensor("wpf", [3, 32, 768], BF16, kind="ExternalInput").ap(),
        "gamma": nc.dram_tensor("gamma", [1], F32, kind="ExternalInput").ap(),
        "y": nc.dram_tensor("y", [C, 64, W], BF16, kind="ExternalOutput").ap(),
    }
    with tile.TileContext(nc, trace_sim=trace_sim) as tc:
        with ExitStack() as _ctx:
            if loop_k is None:
                pools = make_pools(tc, _ctx, big_bufs=min(static_k, 2))
                cst = build_consts(tc, aps, pools)
                for _ in range(static_k):
                    build_body(tc, aps, pools, cst, use_cc)
            else:
                # three rotating bodies per hardware-loop iteration: each
                # body's load/projection phase overlaps its predecessors'
                # conv/store phases
                assert loop_k % 3 == 0
                pools = make_pools(tc, _ctx, big_bufs=3)
                cst = build_consts(tc, aps, pools)
                with tc.For_i(0, loop_k // 3, 1):
                    build_body(tc, aps, pools, cst, use_cc)
                    build_body(tc, aps, pools, cst, use_cc)
                    build_body(tc, aps, pools, cst, use_cc)
    nc.finalize()
    return nc


class SpmdRunner:
    def __init__(self, nc, n_cores):
        install_neuronx_cc_hook()
        self.nc = nc
        self.n_cores = n_cores
        partition_name = nc.partition_id_tensor.name if nc.partition_id_tensor else None
        in_names, out_names, out_avals, zero_outs = [], [], [], []
        for alloc in nc.m.functions[0].allocations:
            if not isinstance(alloc, mybir.MemoryLocationSet):
                continue
            name = alloc.memorylocations[0].name
            if alloc.kind == "ExternalInput":
                if name != partition_name:
                    in_names.append(name)
            elif alloc.kind == "ExternalOutput":
                shape = tuple(alloc.tensor_shape)
                dtype = mybir.dt.np(alloc.dtype)
                out_names.append(name)
                out_avals.append(jax.core.ShapedArray(shape, dtype))
                zero_outs.append(np.zeros(shape, dtype))
        self.in_names, self.out_names = in_names, out_names
        self.out_avals, self.zero_outs = out_avals, zero_outs
        self.n_params = len(in_names)
        all_in = list(in_names) + list(out_names)
        if partition_name is not None:
            all_in.append(partition_name)

        def _body(*args):
            operands = list(args)
            if partition_name is not None:
                operands.append(partition_id_tensor())
            return tuple(_bass_exec_p.bind(
                *operands, out_avals=tuple(out_avals), in_names=tuple(all_in),
                out_names=tuple(out_names), lowering_input_output_aliases=(),
                sim_require_finite=False, sim_require_nnan=False, nc=nc))

        devices = jax.devices()[:n_cores]
        self.mesh = Mesh(np.asarray(devices), ("core",))
        n_outs = len(out_avals)
        in_specs = (PartitionSpec("core"),) * (self.n_params + n_outs)
        out_specs = (PartitionSpec("core"),) * n_outs
        self.sharded = jax.jit(
            shard_map(_body, mesh=self.mesh, in_specs=in_specs,
                      out_specs=out_specs, check_rep=False),
            keep_unused=True)

    def prepare(self, in_maps):
        n = self.n_cores
        concat_in = [
            np.concatenate([np.asarray(in_maps[c][k]) for c in range(n)], axis=0)
            for k in self.in_names
        ]
        concat_zero = [np.zeros((n * z.shape[0], *z.shape[1:]), z.dtype)
                       for z in self.zero_outs]
        sh = NamedSharding(self.mesh, PartitionSpec("core"))
        return [jax.device_put(a, sh) for a in concat_in + concat_zero]

    def run(self, args):
        outs = self.sharded(*args)
        jax.block_until_ready(outs)
        return outs

    def results(self, outs):
        n = self.n_cores
        return [
            {name: np.asarray(outs[i]).reshape(n, *self.out_avals[i].shape)[c]
             for i, name in enumerate(self.out_names)}
            for c in range(n)
        ]


_RUNNER_CACHE = {}


def get_runner(loop_k=None, use_cc=True, static_k=1):
    key = (loop_k, use_cc, static_k)
    if key not in _RUNNER_CACHE:
        _RUNNER_CACHE[key] = SpmdRunner(
            build_nc(loop_k, use_cc, static_k=static_k), N_CORES)
    return _RUNNER_CACHE[key]


def make_in_maps(x, wq, bq, wk, bk, wv, bv, wp, gamma):
    """Shard FULL inputs into 8 per-core input dicts (with flip trick)."""
    B = x.shape[0]
    bf = ml_dtypes.bfloat16
    wqkT = np.ascontiguousarray(
        np.concatenate([wq.T, wk.T], axis=1).reshape(2, 128, 64)).astype(bf)
    wvT = np.ascontiguousarray(wv.T.reshape(2, 128, 32)).astype(bf)
    bqk = np.concatenate([bq, bk]).astype(np.float32)
    bqk8 = np.ascontiguousarray(
        np.tile(bqk[None, None, :], (128, 8, 1)).reshape(128, 512)).astype(bf)
    bvf = bv.astype(np.float32)
    # wpf[dx, d, dy*256 + c] = wp[c, d, dy(_eff), dx]
    wpf_n = np.ascontiguousarray(
        np.transpose(wp, (3, 1, 2, 0)).reshape(3, 32, 768)).astype(bf)
    wp_fl = wp[:, :, ::-1, :]
    wpf_f = np.ascontiguousarray(
        np.transpose(wp_fl, (3, 1, 2, 0)).reshape(3, 32, 768)).astype(bf)
    gam = gamma.astype(np.float32)

    in_maps = []
    for b in range(B):
        top = np.ascontiguousarray(
            x[b, :, 0:HE, :]).reshape(2, 128, NE).astype(bf)
        bot = np.ascontiguousarray(
            x[b, :, H - 1:H - 1 - HE:-1, :]).reshape(2, 128, NE).astype(bf)
        for xec, wpfc in ((top, wpf_n), (bot, wpf_f)):
            in_maps.append(dict(xe=xec, wqkT=wqkT, wvT=wvT, bqk8=bqk8,
                                bv=bvf, wpf=wpfc, gamma=gam))
    return in_maps


def assemble(results):
    """Gather per-core [256, 64, 128] bf16 outputs into f32 [4, 256, 128, 128]."""
    B = len(results) // 2
    y = np.empty((B, C, H, W), np.float32)
    for b in range(B):
        y[b, :, 0:64, :] = results[2 * b]["y"].astype(np.float32)
        y[b, :, 64:128, :] = results[2 * b + 1]["y"][:, ::-1, :].astype(np.float32)
    return y


def kernel(**inputs):
    r = get_runner(None)
    in_maps = make_in_maps(**inputs)
    args = r.prepare(in_maps)
    outs = r.run(args)
    return assemble(r.results(outs))


# revision 69
# speedup vs baseline: 1.0348x; 1.0348x over previous
"""Trainium2 Bass kernel for nn_CAM: channel attention (CAM) block.

y = gamma * gelu(conv3x3(attn(x))) + x   with
  q/k/v = 1x1 conv projections (d = C/8 = 32),
  energy[d,e] = sum_n q[d,n] k[e,n]  (n over all H*W positions),
  attn = softmax(max_e(energy) - energy, axis=e)  (== softmax(-energy)),
  out  = attn @ v.

Sharding: 8 cores, 2 per sample (B=4). Each core handles 64 rows of H plus
one halo row. Bottom-half cores receive a vertically flipped tile (and a
dy-flipped conv weight) so the SPMD program is identical on all cores; the
energy partial sums are combined with a pairwise AllReduce (4 KB).

Kernel pipeline (per core), all bf16 on-chip except f32 PSUM/softmax:
  x bf16 (host pre-cast) -> SBUF                       [4.26 MB HBM load]
  qk^T chunks = x_chunk(stationary).T @ [wq|wk]        [Q^T/K^T direct --
    no DMA transpose; biases added in the PSUM evac via a host-broadcast
    constant, so the energy is exact with no correction terms]
  E[32,32] = sum_chunks Q'^T K'  (exact biased energy, f32 PSUM accum)
  E -> pairwise AllReduce;  V = wv.T @ x + bv -> pa3 center block plus
    two flat +-1-shifted replica copies (SBUF DMAs) DURING the CC window
  softmax(-E) -> attn;  wp' = attn.T @ wp  (fold attention into the conv
    weights; conv then reads the V replicas directly -- no attn@V pass)
  conv3x3 = 3 accumulating K=96 bf16 matmuls per [128,1024] tile,
    dy-major so each weight stationary serves 2 consecutive matmuls
  y = gamma * gelu(conv) + x  (ACT gelu + DVE STT), bf16 store
  (host assembles and upcasts to f32)
"""
import sys

sys.path.insert(0, "/opt/trn_rl_repo")

from contextlib import ExitStack

import numpy as np
import ml_dtypes

import jax
from jax.sharding import Mesh, PartitionSpec, NamedSharding
from jax.experimental.shard_map import shard_map

import concourse.bacc as bacc
import concourse.tile as tile
from concourse import mybir
import concourse.bass as bass
from concourse.bass2jax import (
    _bass_exec_p,
    install_neuronx_cc_hook,
    partition_id_tensor,
)

F32 = mybir.dt.float32
BF16 = mybir.dt.bfloat16
OP = mybir.AluOpType
AF = mybir.ActivationFunctionType

C = 256
D = 32
H = 128
W = 128
HE = 65          # rows per core incl. 1 halo row
NE = HE * W      # 8320
NOWN = 64 * W    # 8192 (rows owned by this core)
NCH = 65         # 128-col chunks incl. halo row chunk
N_CORES = 8
REPLICA_GROUPS = [[0, 1], [2, 3], [4, 5], [6, 7]]


def make_pools(tc, _ctx, big_bufs=1):
    # per-tensor lifetimes: x is read to the very end of a body (bufs=N),
    # pa3 dies at its body's conv (bufs<=3), qk dies after the gram (bufs<=2)
    return dict(
        consts=_ctx.enter_context(tc.tile_pool(name="consts", bufs=1)),
        big=_ctx.enter_context(tc.tile_pool(name="big", bufs=min(big_bufs, 3))),
        pa3p=_ctx.enter_context(tc.tile_pool(name="pa3p", bufs=min(big_bufs, 3))),
        qkp=_ctx.enter_context(tc.tile_pool(name="qkp", bufs=min(big_bufs, 2))),
        work=_ctx.enter_context(tc.tile_pool(name="work", bufs=4)),
        small=_ctx.enter_context(tc.tile_pool(name="small", bufs=2)),
        ps_qk=_ctx.enter_context(tc.tile_pool(name="ps_qk", bufs=3, space="PSUM")),
        ps_e=_ctx.enter_context(tc.tile_pool(name="ps_e", bufs=1, space="PSUM")),
        ps_c=_ctx.enter_context(tc.tile_pool(name="ps_c", bufs=2, space="PSUM")),
        dram=_ctx.enter_context(tc.tile_pool(name="dram", bufs=1, space="DRAM")),
    )


def build_consts(tc, aps, pools):
    """Load all loop-invariant constants once, outside the For_i body."""
    nc = tc.nc
    wqkT, wvT, bqk8, bvv, wpf, gam = (
        aps["wqkT"], aps["wvT"], aps["bqk8"], aps["bv"], aps["wpf"],
        aps["gamma"],
    )
    consts = pools["consts"]
    wqk_sb = consts.tile([128, 2, 64], BF16, tag="wqk")
    wv_sb = consts.tile([128, 2, 32], BF16, tag="wv")
    for c in range(2):
        nc.gpsimd.dma_start(out=wqk_sb[:, c, :], in_=wqkT[c])
        nc.gpsimd.dma_start(out=wv_sb[:, c, :], in_=wvT[c])
    bqk8_sb = consts.tile([128, 8, 64], BF16, tag="bqk8")
    nc.gpsimd.dma_start(out=bqk8_sb[:].rearrange("p a b -> p (a b)"), in_=bqk8)
    bv_sb = consts.tile([32, 1], F32)
    nc.gpsimd.dma_start(
        out=bv_sb[:],
        in_=bass.AP(tensor=bvv.tensor, offset=bvv.offset, ap=[[1, 32], [1, 1]]))
    wpf_sb = consts.tile([32, 3, 768], BF16, tag="wpf")
    for dx in range(3):
        nc.gpsimd.dma_start(out=wpf_sb[:, dx, :], in_=wpf[dx])
    gam_sb = consts.tile([128, 1], BF16)
    nc.gpsimd.dma_start(
        out=gam_sb[:],
        in_=bass.AP(tensor=gam.tensor, offset=gam.offset, ap=[[0, 128], [1, 1]]))
    return dict(wqk_sb=wqk_sb, wv_sb=wv_sb, bqk8_sb=bqk8_sb, bv_sb=bv_sb,
                wpf_sb=wpf_sb, gam_sb=gam_sb)


def build_body(tc, aps, pools, cst, use_cc=True):
    nc = tc.nc
    xe, y = aps["xe"], aps["y"]
    y_f = y.rearrange("c h w -> c (h w)")            # [256, 8192] bf16

    big, work, small = pools["big"], pools["work"], pools["small"]
    ps_qk, ps_e, ps_c, dram = (pools["ps_qk"], pools["ps_e"], pools["ps_c"],
                               pools["dram"])
    wqk_sb, wv_sb, bqk8_sb, bv_sb, wpf_sb, gam_sb = (
        cst["wqk_sb"], cst["wv_sb"], cst["bqk8_sb"], cst["bv_sb"],
        cst["wpf_sb"], cst["gam_sb"])

    # ---- big SBUF tensors ----
    x_sb = big.tile([128, 2, NE], BF16)              # both channel halves
    qk_sb = pools["qkp"].tile([128, 64, 64], BF16)   # [n128, chunk, q|k]
    pa3 = pools["pa3p"].tile([96, 66, 130], BF16)    # V replicated w-shifted x3

    # ---- x load (bf16, sync + scalar queues), 8 groups of 1024 cols ----
    for g in range(8):
        s = g * 1024
        nc.sync.dma_start(out=x_sb[:, 0, s:s + 1024], in_=xe[0, :, s:s + 1024])
        nc.scalar.dma_start(out=x_sb[:, 1, s:s + 1024], in_=xe[1, :, s:s + 1024])
    nc.sync.dma_start(out=x_sb[:, 0, 8192:NE], in_=xe[0, :, 8192:NE])
    nc.scalar.dma_start(out=x_sb[:, 1, 8192:NE], in_=xe[1, :, 8192:NE])

    # pa3 zero padding: top row (h=-1) + center-block pad columns
    nc.vector.memset(pa3[:, 0, :], 0.0)
    nc.vector.memset(pa3[32:64, :, 0], 0.0)
    nc.vector.memset(pa3[32:64, :, 129], 0.0)

    # ---- qk^T chunks + energy accumulation ----
    # qk^T[n, ch] for chunk k: x_chunk [128c, 128n] stationary, wqk streamed;
    # biases folded in during the PSUM evac via a host-broadcast constant.
    e_ps = ps_e.tile([32, 32], F32, tag="eps")
    for g in range(8):
        qp = ps_qk.tile([128, 512], F32, tag="qk")
        for k in range(8):
            ch = 8 * g + k
            sl = slice(ch * 128, ch * 128 + 128)
            nc.tensor.matmul(qp[:, 64 * k:64 * k + 64], x_sb[:, 0, sl],
                             wqk_sb[:, 0, :], start=True, stop=False)
            nc.tensor.matmul(qp[:, 64 * k:64 * k + 64], x_sb[:, 1, sl],
                             wqk_sb[:, 1, :], start=False, stop=True)
        nc.vector.tensor_tensor(
            out=qk_sb[:, 8 * g:8 * g + 8, :],
            in0=qp[:].rearrange("p (k c) -> p k c", c=64),
            in1=bqk8_sb[:], op=OP.add)
        for k in range(8):
            ch = 8 * g + k
            nc.tensor.matmul(e_ps[:], qk_sb[:, ch, 0:32],
                             qk_sb[:, ch, 32:64],
                             start=(ch == 0), stop=(ch == 63))

    e_sb = small.tile([32, 32], F32, tag="esb")
    nc.vector.tensor_copy(out=e_sb[:], in_=e_ps[:])

    # ---- V projection -> pa3 center block (fills the CC window) ----
    nv = (NE + 511) // 512  # 17
    for i in range(nv):
        s = i * 512
        w = min(512, NE - s)
        nh = w // 128
        r0 = s // 128
        vp = ps_qk.tile([32, 512], F32, tag="qk")
        nc.tensor.matmul(vp[:, :w], wv_sb[:, 0, :], x_sb[:, 0, s:s + w],
                         start=True, stop=False)
        nc.tensor.matmul(vp[:, :w], wv_sb[:, 1, :], x_sb[:, 1, s:s + w],
                         start=False, stop=True)
        dst = pa3[32:64, 1 + r0:1 + r0 + nh, 1:129]
        src = vp[:, :w].rearrange("p (h w) -> p h w", w=128)
        if i % 2 == 0:
            nc.vector.tensor_scalar(out=dst, in0=src, scalar1=bv_sb[:],
                                    scalar2=None, op0=OP.add)
        else:
            nc.scalar.activation(out=dst, in_=src, func=AF.Identity,
                                 bias=bv_sb[:], scale=1.0)

    # ---- AllReduce energy across the sample pair ----
    E_sb = small.tile([32, 32], F32, tag="Esb")
    if use_cc:
        ein = dram.tile([32, 32], F32)
        eout = dram.tile([32, 32], F32)
        nc.gpsimd.dma_start(out=ein[:], in_=e_sb[:])
        nc.gpsimd.collective_compute(
            "AllReduce", OP.add, replica_groups=REPLICA_GROUPS,
            ins=[ein.opt()], outs=[eout.opt()])
        nc.sync.dma_start(out=E_sb[:], in_=eout[:])
    else:
        nc.gpsimd.tensor_copy(out=E_sb[:], in_=e_sb[:])

    # replicate the two w-shifted blocks as flat +-1-element shifted copies
    # (the zero pad columns make the row-boundary bleed exactly correct);
    # quartered so each segment launches as soon as its V rows land
    ctr = pa3[32:64, :, :].rearrange("p a b -> p (a b)")
    bl0 = pa3[0:32, :, :].rearrange("p a b -> p (a b)")
    bl2 = pa3[64:96, :, :].rearrange("p a b -> p (a b)")
    for r0, r1 in ((0, 17), (17, 34), (34, 50), (50, 66)):
        a, b = r0 * 130, r1 * 130
        if r1 == 66:
            b = 8579
        nc.sync.dma_start(out=bl0[:, a + 1:b + 1], in_=ctr[:, a:b])
        nc.scalar.dma_start(out=bl2[:, a:b], in_=ctr[:, a + 1:b + 1])

    # ---- softmax over e of -E, stable via min ----
    rmin = small.tile([32, 1], F32, tag="rmin")
    nc.vector.tensor_reduce(out=rmin[:], in_=E_sb[:], axis=mybir.AxisListType.X,
                            op=OP.min)
    p_sb = small.tile([32, 32], F32, tag="psb")
    nc.scalar.activation(out=p_sb[:], in_=E_sb[:], func=AF.Exp, scale=-1.0,
                         bias=rmin[:])
    ssum = small.tile([32, 1], F32, tag="ssum")
    nc.vector.reduce_sum(out=ssum[:], in_=p_sb[:], axis=mybir.AxisListType.X)
    rs = small.tile([32, 1], F32, tag="rs")
    nc.vector.reciprocal(out=rs[:], in_=ssum[:])
    attn_sb = small.tile([32, 32], BF16, tag="attn")
    nc.vector.tensor_scalar(out=attn_sb[:], in0=p_sb[:], scalar1=rs[:],
                            scalar2=None, op0=OP.mult)

    # ---- fold attention into conv weights: wp'[(dx,e),(dy,c)] ----
    wpp_sb = small.tile([96, 3, 256], BF16, tag="wpp")
    for dx in range(3):
        fp = ps_c.tile([32, 768], F32, tag="mm")
        nc.tensor.matmul(fp[:, 0:512], attn_sb[:], wpf_sb[:, dx, 0:512],
                         start=True, stop=True)
        nc.tensor.matmul(fp[:, 512:768], attn_sb[:], wpf_sb[:, dx, 512:768],
                         start=True, stop=True)
        dst = wpp_sb[32 * dx:32 * dx + 32, :, :].rearrange("p a b -> p (a b)")
        if dx == 1:
            nc.scalar.activation(out=dst, in_=fp[:], func=AF.Copy)
        else:
            nc.vector.tensor_copy(out=dst, in_=fp[:])

    # ---- conv 3x3 (bf16) + exact gelu + gamma*out + x, then store ----
    # dy-major matmul order within each [128,1024] tile: each conv weight
    # stationary is reused for 2 consecutive 512-col matmuls.
    for half in range(2):
        for tg in range(8):
            cp = ps_c.tile([128, 1024], F32, tag="mm")
            for dy in range(3):
                for tq in range(2):
                    t = 2 * tg + tq
                    nc.tensor.matmul(
                        cp[:, 512 * tq:512 * (tq + 1)],
                        wpp_sb[:, dy, half * 128:(half + 1) * 128],
                        pa3[:, 4 * t + dy:4 * t + dy + 4, 1:129],
                        start=(dy == 0), stop=(dy == 2))
            yt = work.tile([128, 1024], BF16, tag="yt")
            nc.scalar.activation(out=yt[:], in_=cp[:], func=AF.Gelu)
            t0 = 2 * tg * 512
            yo = work.tile([128, 1024], BF16, tag="yo")
            nc.vector.scalar_tensor_tensor(
                out=yo[:], in0=yt[:], scalar=gam_sb[:],
                in1=x_sb[:, half, t0:t0 + 1024],
                op0=OP.mult, op1=OP.add)
            st_eng = nc.sync if tg % 2 == 0 else nc.gpsimd
            st_eng.dma_start(
                out=y_f[half * 128:(half + 1) * 128, t0:t0 + 1024], in_=yo[:])


def build_nc(loop_k=None, use_cc=True, trace_sim=False, static_k=1):
    nc = bacc.Bacc("TRN2", target_bir_lowering=False, debug=False,
                   num_devices=N_CORES)
    aps = {
        "xe": nc.dram_tensor("xe", [2, 128, NE], BF16, kind="ExternalInput").ap(),
        "wqkT": nc.dram_tensor("wqkT", [2, 128, 64], BF16, kind="ExternalInput").ap(),
        "wvT": nc.dram_tensor("wvT", [2, 128, 32], BF16, kind="ExternalInput").ap(),
        "bqk8": nc.dram_tensor("bqk8", [128, 512], BF16, kind="ExternalInput").ap(),
        "bv": nc.dram_tensor("bv", [D], F32, kind="ExternalInput").ap(),
        "wpf": nc.dram_tensor("wpf", [3, 32, 768], BF16, kind="ExternalInput").ap(),
        "gamma": nc.dram_tensor("gamma", [1], F32, kind="ExternalInput").ap(),
        "y": nc.dram_tensor("y", [C, 64, W], BF16, kind="ExternalOutput").ap(),
    }
    with tile.TileContext(nc, trace_sim=trace_sim) as tc:
        with ExitStack() as _ctx:
            if loop_k is None:
                pools = make_pools(tc, _ctx, big_bufs=min(static_k, 2))
                cst = build_consts(tc, aps, pools)
                for _ in range(static_k):
                    build_body(tc, aps, pools, cst, use_cc)
            else:
                # four rotating bodies per hardware-loop iteration: each
                # body's load/projection phase overlaps its predecessors'
                # conv/store phases (x lookahead capped at 3 buffers)
                assert loop_k % 4 == 0
                pools = make_pools(tc, _ctx, big_bufs=4)
                cst = build_consts(tc, aps, pools)
                with tc.For_i(0, loop_k // 4, 1):
                    for _ in range(4):
                        build_body(tc, aps, pools, cst, use_cc)
    nc.finalize()
    return nc


class SpmdRunner:
    def __init__(self, nc, n_cores):
        install_neuronx_cc_hook()
        self.nc = nc
        self.n_cores = n_cores
        partition_name = nc.partition_id_tensor.name if nc.partition_id_tensor else None
        in_names, out_names, out_avals, zero_outs = [], [], [], []
        for alloc in nc.m.functions[0].allocations:
            if not isinstance(alloc, mybir.MemoryLocationSet):
                continue
            name = alloc.memorylocations[0].name
            if alloc.kind == "ExternalInput":
                if name != partition_name:
                    in_names.append(name)
            elif alloc.kind == "ExternalOutput":
                shape = tuple(alloc.tensor_shape)
                dtype = mybir.dt.np(alloc.dtype)
                out_names.append(name)
                out_avals.append(jax.core.ShapedArray(shape, dtype))
                zero_outs.append(np.zeros(shape, dtype))
        self.in_names, self.out_names = in_names, out_names
        self.out_avals, self.zero_outs = out_avals, zero_outs
        self.n_params = len(in_names)
        all_in = list(in_names) + list(out_names)
        if partition_name is not None:
            all_in.append(partition_name)

        def _body(*args):
            operands = list(args)
            if partition_name is not None:
                operands.append(partition_id_tensor())
            return tuple(_bass_exec_p.bind(
                *operands, out_avals=tuple(out_avals), in_names=tuple(all_in),
                out_names=tuple(out_names), lowering_input_output_aliases=(),
                sim_require_finite=False, sim_require_nnan=False, nc=nc))

        devices = jax.devices()[:n_cores]
        self.mesh = Mesh(np.asarray(devices), ("core",))
        n_outs = len(out_avals)
        in_specs = (PartitionSpec("core"),) * (self.n_params + n_outs)
        out_specs = (PartitionSpec("core"),) * n_outs
        self.sharded = jax.jit(
            shard_map(_body, mesh=self.mesh, in_specs=in_specs,
                      out_specs=out_specs, check_rep=False),
            keep_unused=True)

    def prepare(self, in_maps):
        n = self.n_cores
        concat_in = [
            np.concatenate([np.asarray(in_maps[c][k]) for c in range(n)], axis=0)
            for k in self.in_names
        ]
        concat_zero = [np.zeros((n * z.shape[0], *z.shape[1:]), z.dtype)
                       for z in self.zero_outs]
        sh = NamedSharding(self.mesh, PartitionSpec("core"))
        return [jax.device_put(a, sh) for a in concat_in + concat_zero]

    def run(self, args):
        outs = self.sharded(*args)
        jax.block_until_ready(outs)
        return outs

    def results(self, outs):
        n = self.n_cores
        return [
            {name: np.asarray(outs[i]).reshape(n, *self.out_avals[i].shape)[c]
             for i, name in enumerate(self.out_names)}
            for c in range(n)
        ]


_RUNNER_CACHE = {}


def get_runner(loop_k=None, use_cc=True, static_k=1):
    key = (loop_k, use_cc, static_k)
    if key not in _RUNNER_CACHE:
        _RUNNER_CACHE[key] = SpmdRunner(
            build_nc(loop_k, use_cc, static_k=static_k), N_CORES)
    return _RUNNER_CACHE[key]


def make_in_maps(x, wq, bq, wk, bk, wv, bv, wp, gamma):
    """Shard FULL inputs into 8 per-core input dicts (with flip trick)."""
    B = x.shape[0]
    bf = ml_dtypes.bfloat16
    wqkT = np.ascontiguousarray(
        np.concatenate([wq.T, wk.T], axis=1).reshape(2, 128, 64)).astype(bf)
    wvT = np.ascontiguousarray(wv.T.reshape(2, 128, 32)).astype(bf)
    bqk = np.concatenate([bq, bk]).astype(np.float32)
    bqk8 = np.ascontiguousarray(
        np.tile(bqk[None, None, :], (128, 8, 1)).reshape(128, 512)).astype(bf)
    bvf = bv.astype(np.float32)
    # wpf[dx, d, dy*256 + c] = wp[c, d, dy(_eff), dx]
    wpf_n = np.ascontiguousarray(
        np.transpose(wp, (3, 1, 2, 0)).reshape(3, 32, 768)).astype(bf)
    wp_fl = wp[:, :, ::-1, :]
    wpf_f = np.ascontiguousarray(
        np.transpose(wp_fl, (3, 1, 2, 0)).reshape(3, 32, 768)).astype(bf)
    gam = gamma.astype(np.float32)

    in_maps = []
    for b in range(B):
        top = np.ascontiguousarray(
            x[b, :, 0:HE, :]).reshape(2, 128, NE).astype(bf)
        bot = np.ascontiguousarray(
            x[b, :, H - 1:H - 1 - HE:-1, :]).reshape(2, 128, NE).astype(bf)
        for xec, wpfc in ((top, wpf_n), (bot, wpf_f)):
            in_maps.append(dict(xe=xec, wqkT=wqkT, wvT=wvT, bqk8=bqk8,
                                bv=bvf, wpf=wpfc, gamma=gam))
    return in_maps


def assemble(results):
    """Gather per-core [256, 64, 128] bf16 outputs into f32 [4, 256, 128, 128]."""
    B = len(results) // 2
    y = np.empty((B, C, H, W), np.float32)
    for b in range(B):
        y[b, :, 0:64, :] = results[2 * b]["y"].astype(np.float32)
        y[b, :, 64:128, :] = results[2 * b + 1]["y"][:, ::-1, :].astype(np.float32)
    return y


def kernel(**inputs):
    r = get_runner(None)
    in_maps = make_in_maps(**inputs)
    args = r.prepare(in_maps)
    outs = r.run(args)
    return assemble(r.results(outs))
